# revision 21
# baseline (speedup 1.0000x reference)
"""Trainium2 Bass kernel for nn_BatchDelayProcessor.

Computes, per batch row (B=64, T=441000, D=22050 delay, 20 blocks):
    delayed[t] = 0                          , t < D
    delayed[t] = x[t-D] + 0.3*delayed[t-D]  , t >= D
    out[t]     = 0.5*x[t] + 0.5*delayed[t]

Block recurrence (blocks of D samples):  d_{k+1} = x_k + 0.3*d_k, d_0 = 0;
out_k = 0.5*x_k + 0.5*d_k.  With a scaled carry c_k = 0.5*d_k:
    h_k     = 0.5 * x_k          (ACT engine, copy-with-scale)
    out_k   = h_k + c_k          (DVE tensor_tensor add)
    g_k     = -0.7 * c_k         (ACT)
    c_{k+1} = g_k + out_k        (DVE)     [1 - 0.7 == 0.3 exactly in f32]

Sharding: data-parallel over batch — 8 rows per NeuronCore, 8 cores, no
communication.  Per-core layout: each block (8 rows x 22050) is viewed as
(120 partitions x 1470 f32): partition (r*15+s) holds row r, sub-slice s
(1470 contiguous samples = 5880 B per DMA descriptor row).

Raw Bass (not Tile): Tile's semaphore assignment put 3 sync waits on one
compute instruction, which overflows the walrus codegen per-instruction
sync-wait encoding ("Too many sync wait commands").  Here each wait is a
standalone sequencer wait_ge, and the pipeline is hand-scheduled:
  SP sequencer: DMA-in block k+NX / DMA-out block k  (HWDGE)
  ACT:          h_k, g_k
  DVE:          out_k, c_{k+1}
with ring buffers (x: NX tiles, out: NO tiles, h/g/c: 2 tiles each).
"""

from contextlib import ExitStack

import numpy as np

import concourse.bass as bass
import concourse.mybir as mybir
from concourse.bass_utils import run_bass_kernel_spmd

B, T = 64, 441000
D, NBLK = 22050, 20
NCORES = 8
ROWS = B // NCORES          # 8 rows per core
SPLITS = 15                 # 22050 = 15 * 1470
FREE = D // SPLITS          # 1470
P = ROWS * SPLITS           # 120 partitions

NX = 12                     # x-tile ring (DMA-in lookahead > 8 load queues)
NO = 12                     # out-tile ring
NH = 2                      # h ring
NG = 2                      # g ring
NC = 2                      # carry ping-pong

F32 = mybir.dt.float32


def build_nc() -> bass.Bass:
    nc = bass.Bass(trn_type="TRN2")
    x = nc.declare_dram_parameter("x", [ROWS, T], F32, isOutput=False)
    y = nc.declare_dram_parameter("y", [ROWS, T], F32, isOutput=True)
    # (block, row, split, free)
    xv = x.rearrange("r (k s f) -> k r s f", k=NBLK, s=SPLITS)
    yv = y.rearrange("r (k s f) -> k r s f", k=NBLK, s=SPLITS)

    with ExitStack() as ctx:
        block = ctx.enter_context(nc.Block())
        xbuf = ctx.enter_context(nc.sbuf_tensor("xbuf", [P, NX * FREE], F32))
        obuf = ctx.enter_context(nc.sbuf_tensor("obuf", [P, NO * FREE], F32))
        hbuf = ctx.enter_context(nc.sbuf_tensor("hbuf", [P, NH * FREE], F32))
        gbuf = ctx.enter_context(nc.sbuf_tensor("gbuf", [P, NG * FREE], F32))
        cbuf = ctx.enter_context(nc.sbuf_tensor("cbuf", [P, NC * FREE], F32))
        # Per-ring-slot DMA sems: a slot's next DMA is issued only after the
        # sequencer re-observed the slot sem at its current value, so the
        # async SDMA increments on one sem are never concurrent (same
        # protocol as Tile's DMAHW lanes).
        s_in = [
            ctx.enter_context(nc.semaphore(f"s_in{j}")) for j in range(NX)
        ]
        s_out = [
            ctx.enter_context(nc.semaphore(f"s_out{j}")) for j in range(NO)
        ]
        s_act = ctx.enter_context(nc.semaphore("s_act"))
        s_dve = ctx.enter_context(nc.semaphore("s_dve"))

        def slot(buf, k, n):
            j = k % n
            return buf[:, j * FREE : (j + 1) * FREE]

        # DMA pairs the (8,15,1470) DRAM view with the (120,1470) SBUF slot:
        # traversal orders match since partition p = r*15 + s.
        slot3d = slot

        # Completion-count conventions:
        #   s_in[j] : 16*(k//NX + 1) after DMA-in of block k (j = k%NX)
        #   s_out[j]: 16*(k//NO + 1) after DMA-out of block k (j = k%NO)
        #   s_act   : 2k+1 after h_k, 2k+2 after g_k (g_19 never emitted)
        #   s_dve   : 1 after memset, 2k+2 after out_k, 2k+3 after c_{k+1}

        # DMA-ins on the SP HWDGE ring, DMA-outs on the GpSimd SWDGE ring
        # (one direction per ring; measured faster than mixing).  Loads
        # round-robin over 8 HW load queues, each latency-bound per
        # descriptor, so keep > 8 loads in flight.
        @block.sync
        def _(sync):
            for k in range(NBLK):
                if k >= NX:
                    # WAR: xbuf slot k%NX last read by h_{k-NX}
                    sync.wait_ge(s_act, 2 * (k - NX) + 1)
                    # slot sem at its current value (race-free async incs)
                    sync.wait_ge(s_in[k % NX], 16 * (k // NX))
                sync.dma_start(out=slot3d(xbuf, k, NX), in_=xv[k]).then_inc(
                    s_in[k % NX], 16
                )

        @block.gpsimd
        def _(gpsimd):
            for k in range(NBLK):
                gpsimd.wait_ge(s_dve, 2 * k + 2)  # out_k ready
                if k >= NO:
                    gpsimd.wait_ge(s_out[k % NO], 16 * (k // NO))
                gpsimd.dma_start(out=yv[k], in_=slot3d(obuf, k, NO)).then_inc(
                    s_out[k % NO], 16
                )

        @block.scalar
        def _(scalar):
            for k in range(NBLK):
                scalar.wait_ge(s_in[k % NX], 16 * (k // NX + 1))  # x_k loaded
                if k >= NH:
                    # WAR: hbuf slot k%NH last read by out_{k-NH}
                    scalar.wait_ge(s_dve, 2 * (k - NH) + 2)
                nc.scalar.mul(slot(hbuf, k, NH), slot(xbuf, k, NX), 0.5).then_inc(
                    s_act, 1
                )
                if k < NBLK - 1:
                    # c_k ready (memset for k=0, c-update of iter k-1 else);
                    # also covers WAR on gbuf slot k%NG (read by c_{k-1})
                    scalar.wait_ge(s_dve, 2 * k + 1)
                    nc.scalar.mul(
                        slot(gbuf, k, NG), slot(cbuf, k, NC), -0.7
                    ).then_inc(s_act, 1)

        @block.vector
        def _(vector):
            nc.vector.memset(slot(cbuf, 0, NC), 0.0).then_inc(s_dve, 1)
            for k in range(NBLK):
                vector.wait_ge(s_act, 2 * k + 1)  # h_k ready
                # DVE writes drain async: same-engine RAW on c_k needs a wait
                vector.wait_ge(s_dve, 2 * k + 1)  # c_k drained (memset @ k=0)
                if k >= NO:
                    # WAR: obuf slot k%NO last read by DMA-out of k-NO
                    vector.wait_ge(s_out[k % NO], 16 * (k // NO))
                nc.vector.tensor_add(
                    out=slot(obuf, k, NO),
                    in0=slot(hbuf, k, NH),
                    in1=slot(cbuf, k, NC),
                ).then_inc(s_dve, 1)
                if k < NBLK - 1:
                    vector.wait_ge(s_act, 2 * k + 2)  # g_k ready
                    vector.wait_ge(s_dve, 2 * k + 2)  # out_k drained
                    nc.vector.tensor_add(
                        out=slot(cbuf, k + 1, NC),
                        in0=slot(gbuf, k, NG),
                        in1=slot(obuf, k, NO),
                    ).then_inc(s_dve, 1)

    return nc


_NC_CACHE = None


def _get_nc() -> bass.Bass:
    global _NC_CACHE
    if _NC_CACHE is None:
        _NC_CACHE = build_nc()
    return _NC_CACHE


def _shard(x: np.ndarray) -> list[dict[str, np.ndarray]]:
    x = np.ascontiguousarray(np.asarray(x, dtype=np.float32))
    assert x.shape == (B, T), x.shape
    return [
        {"x": np.ascontiguousarray(x[i * ROWS : (i + 1) * ROWS])}
        for i in range(NCORES)
    ]


def kernel(x: np.ndarray) -> np.ndarray:
    nc = _get_nc()
    res = run_bass_kernel_spmd(nc, _shard(x), core_ids=list(range(NCORES)))
    return np.concatenate([r["y"] for r in res.results], axis=0)


def kernel_profiled(x: np.ndarray):
    """Like kernel() but with NTFF tracing; returns (out, BassKernelResults)."""
    nc = _get_nc()
    res = run_bass_kernel_spmd(
        nc, _shard(x), core_ids=list(range(NCORES)), trace=True
    )
    out = np.concatenate([r["y"] for r in res.results], axis=0)
    return out, res


# revision 27
# speedup vs baseline: 1.0555x; 1.0555x over previous
"""Trainium2 Bass kernel for nn_BatchDelayProcessor.

Computes, per batch row (B=64, T=441000, D=22050 delay, 20 blocks):
    delayed[t] = 0                          , t < D
    delayed[t] = x[t-D] + 0.3*delayed[t-D]  , t >= D
    out[t]     = 0.5*x[t] + 0.5*delayed[t]

Block recurrence (blocks of D samples):  d_{k+1} = x_k + 0.3*d_k, d_0 = 0;
out_k = 0.5*x_k + 0.5*d_k.  With a scaled carry c_k = 0.5*d_k, two fused
scalar_tensor_tensor ops on the vector engine per block:
    out_k   = (x_k * 0.5)  + c_k
    c_{k+1} = (c_k * -0.7) + out_k         [1 - 0.7 == 0.3 exactly in f32]

Sharding: data-parallel over batch — 8 rows per NeuronCore, 8 cores, no
communication.  Per-core layout: each block (8 rows x 22050) is viewed as
(120 partitions x 1470 f32): partition (r*15+s) holds row r, sub-slice s
(1470 contiguous samples = 5880 B per DMA descriptor row).

Raw Bass (not Tile): Tile's semaphore assignment put 3 sync waits on one
compute instruction, which overflows the walrus codegen per-instruction
sync-wait encoding ("Too many sync wait commands").  Here each wait is a
standalone sequencer wait_ge, and the pipeline is hand-scheduled.

Engine split (HWDGE descriptor generation is the load-side bottleneck at
~5us per 120-descriptor load, so loads use BOTH HWDGE rings):
  SP sequencer:  DMA-in of even blocks   (HWDGE ring 0)
  ACT sequencer: DMA-in of odd blocks    (HWDGE ring 1, no compute)
  GpSimd:        DMA-out of all blocks   (SWDGE)
  DVE:           the two STT ops per block
with ring buffers (x: NX tiles, out: NO tiles, carry: 2 tiles).
"""

from contextlib import ExitStack

import numpy as np

import concourse.bass as bass
import concourse.mybir as mybir
from concourse.bass_utils import run_bass_kernel_spmd

B, T = 64, 441000
D, NBLK = 22050, 20
NCORES = 8
ROWS = B // NCORES          # 8 rows per core
SPLITS = 15                 # 22050 = 15 * 1470
FREE = D // SPLITS          # 1470
P = ROWS * SPLITS           # 120 partitions

NX = 12                     # x-tile ring (DMA-in lookahead > 8 load queues)
NO = 12                     # out-tile ring
NC = 2                      # carry ping-pong

F32 = mybir.dt.float32


def build_nc() -> bass.Bass:
    nc = bass.Bass(trn_type="TRN2")
    x = nc.declare_dram_parameter("x", [ROWS, T], F32, isOutput=False)
    y = nc.declare_dram_parameter("y", [ROWS, T], F32, isOutput=True)
    # (block, row, split, free)
    xv = x.rearrange("r (k s f) -> k r s f", k=NBLK, s=SPLITS)
    yv = y.rearrange("r (k s f) -> k r s f", k=NBLK, s=SPLITS)

    with ExitStack() as ctx:
        block = ctx.enter_context(nc.Block())
        xbuf = ctx.enter_context(nc.sbuf_tensor("xbuf", [P, NX * FREE], F32))
        obuf = ctx.enter_context(nc.sbuf_tensor("obuf", [P, NO * FREE], F32))
        cbuf = ctx.enter_context(nc.sbuf_tensor("cbuf", [P, NC * FREE], F32))
        # Per-ring-slot DMA sems: a slot's next DMA is issued only after the
        # sequencer re-observed the slot sem at its current value, so the
        # async SDMA increments on one sem are never concurrent (same
        # protocol as Tile's DMAHW lanes).
        s_in = [
            ctx.enter_context(nc.semaphore(f"s_in{j}")) for j in range(NX)
        ]
        s_out = [
            ctx.enter_context(nc.semaphore(f"s_out{j}")) for j in range(NO)
        ]
        s_dve = ctx.enter_context(nc.semaphore("s_dve"))

        def slot(buf, k, n):
            j = k % n
            return buf[:, j * FREE : (j + 1) * FREE]

        # DMA pairs the (8,15,1470) DRAM view with the (120,1470) SBUF slot:
        # traversal orders match since partition p = r*15 + s.
        slot3d = slot

        # Completion-count conventions:
        #   s_in[j] : 16*(k//NX + 1) after DMA-in of block k (j = k%NX)
        #   s_out[j]: 16*(k//NO + 1) after DMA-out of block k (j = k%NO)
        #   s_dve   : 1 after memset, 2k+2 after out_k, 2k+3 after c_{k+1}
        # NX/NO are even, so one slot's successive DMAs stay on one
        # sequencer and its slot-sem updates stay ordered.

        def emit_in(eng, k):
            if k >= NX:
                # WAR: xbuf slot k%NX last read by out_{k-NX}'s STT
                eng.wait_ge(s_dve, 2 * (k - NX) + 2)
                # slot sem at its current value (race-free async incs)
                eng.wait_ge(s_in[k % NX], 16 * (k // NX))
            eng.dma_start(out=slot3d(xbuf, k, NX), in_=xv[k]).then_inc(
                s_in[k % NX], 16
            )

        @block.sync
        def _(sync):
            for k in range(0, NBLK, 2):
                emit_in(sync, k)

        @block.scalar
        def _(scalar):
            for k in range(1, NBLK, 2):
                emit_in(scalar, k)

        @block.gpsimd
        def _(gpsimd):
            for k in range(NBLK):
                gpsimd.wait_ge(s_dve, 2 * k + 2)  # out_k ready
                if k >= NO:
                    gpsimd.wait_ge(s_out[k % NO], 16 * (k // NO))
                gpsimd.dma_start(out=yv[k], in_=slot3d(obuf, k, NO)).then_inc(
                    s_out[k % NO], 16
                )

        @block.vector
        def _(vector):
            nc.vector.memset(slot(cbuf, 0, NC), 0.0).then_inc(s_dve, 1)
            for k in range(NBLK):
                vector.wait_ge(s_in[k % NX], 16 * (k // NX + 1))  # x_k loaded
                # DVE writes drain async: same-engine RAW on c_k needs a wait
                vector.wait_ge(s_dve, 2 * k + 1)  # c_k drained (memset @ k=0)
                if k >= NO:
                    # WAR: obuf slot k%NO last read by DMA-out of k-NO
                    vector.wait_ge(s_out[k % NO], 16 * (k // NO))
                # out_k = (x_k * 0.5) + c_k
                nc.vector.scalar_tensor_tensor(
                    out=slot(obuf, k, NO),
                    in0=slot(xbuf, k, NX),
                    scalar=0.5,
                    in1=slot(cbuf, k, NC),
                    op0=mybir.AluOpType.mult,
                    op1=mybir.AluOpType.add,
                ).then_inc(s_dve, 1)
                if k < NBLK - 1:
                    vector.wait_ge(s_dve, 2 * k + 2)  # out_k drained
                    # c_{k+1} = (c_k * -0.7) + out_k
                    nc.vector.scalar_tensor_tensor(
                        out=slot(cbuf, k + 1, NC),
                        in0=slot(cbuf, k, NC),
                        scalar=-0.7,
                        in1=slot(obuf, k, NO),
                        op0=mybir.AluOpType.mult,
                        op1=mybir.AluOpType.add,
                    ).then_inc(s_dve, 1)

    return nc


_NC_CACHE = None


def _get_nc() -> bass.Bass:
    global _NC_CACHE
    if _NC_CACHE is None:
        _NC_CACHE = build_nc()
    return _NC_CACHE


def _shard(x: np.ndarray) -> list[dict[str, np.ndarray]]:
    x = np.ascontiguousarray(np.asarray(x, dtype=np.float32))
    assert x.shape == (B, T), x.shape
    return [
        {"x": np.ascontiguousarray(x[i * ROWS : (i + 1) * ROWS])}
        for i in range(NCORES)
    ]


def kernel(x: np.ndarray) -> np.ndarray:
    nc = _get_nc()
    res = run_bass_kernel_spmd(nc, _shard(x), core_ids=list(range(NCORES)))
    return np.concatenate([r["y"] for r in res.results], axis=0)


def kernel_profiled(x: np.ndarray):
    """Like kernel() but with NTFF tracing; returns (out, BassKernelResults)."""
    nc = _get_nc()
    res = run_bass_kernel_spmd(
        nc, _shard(x), core_ids=list(range(NCORES)), trace=True
    )
    out = np.concatenate([r["y"] for r in res.results], axis=0)
    return out, res
